# revision 31
# baseline (speedup 1.0000x reference)
"""Trainium2 Bass kernel for BasicEuclideanDistModel (gnn_message_passing).

Math:
  result = sum_e (beta - ||dz_e + dv_e t_e||)
           - dt * sum_{i<j, s} exp(beta - ||z_i(t_s) - z_j(t_s)||)

Device strategy (8 cores, data parallel):
  * Non-event term: full NxN pairwise distances (halved on host).
    d^2(i,j,s) = F_i(s) . G_j  (K=8 inner product, G time-independent).
    One [8,128]x[8,2048] matmul (fp32r) per (i-tile, s) computes the
    d^2 supertile; DVE relu clamps rounding negatives, ACT computes
    sqrt then exp(-d) with fused row sums.  Each core owns 2 of the
    16 i-tiles, all j, all 10 samples.
  * Event term, split across two independent engines working in
    parallel (events of one u-node always stay together):
    - gpsimd share: d^2(u,v,t) = sum_k A_k(u) B_k(t) C_k(v), a
      14-channel trilinear decomposition with B_k in {1,t,t^2}.
      Events form 8 groups (one per Q7 tile); partition 16g+k holds
      channel k.  ONE ap_gather (SBUF gather, ~27.5ns/idx/core,
      shared index list per group) fetches A_k per segment and C_k
      per event from a [128, N, 2] bf16 channel table; DVE forms
      P = A*C*T (host T = B_k(t)*mask), PE reduces channels with a
      block-ones stationary, one ACT sqrt row-sum -> acc col 20.
    - SWDGE share: baseline scheme -- events grouped by u into
      segments laid out [128, SPD, SLOTD]; dma_gather fetches 256B
      rows (u per segment, v per slot, ~3.8ns/desc aggregate); DVE
      distance algebra, ACT sqrt row-sum -> acc col 21.  Pad slots
      use v=u, t=0 (exactly 0 contribution).
  * beta enters only as a scalar factor / offset -> folded in on host.
  Host combines 8 cores' [128, 24] partial-sum tensors (pure unshard/
  reduction of partials).
"""

import os
import numpy as np


def _import_concourse():
    try:
        import concourse  # noqa: F401
    except ImportError:
        import sys

        for p in ("/opt/trn_rl_repo", "/root/.axon_site/_ro/trn_rl_repo"):
            if os.path.isdir(p) and p not in sys.path:
                sys.path.insert(0, p)


_import_concourse()

from contextlib import ExitStack  # noqa: E402

import concourse.bacc as bacc  # noqa: E402
import concourse.mybir as mybir  # noqa: E402
import concourse.tile as tile  # noqa: E402
from concourse.tile_rust import add_dep_helper  # noqa: E402

N = 2048          # nodes
S = 10            # Riemann samples
NCORES = 8
ITILES = 2        # 128-row i-tiles per core
EV_PER_CORE = 200000 // NCORES       # real events per core

# ---- gpsimd (ap_gather) event share: the highest-count u-nodes ----
NG = 8            # groups (one per Q7 tile of 16 partitions)
SLOT_G = 6        # event slots per segment
SEG_G = 208       # segments per group
EV_G = SEG_G * SLOT_G                # 1248 event slots per group
NIDX = SEG_G + EV_G                  # 1456 gather indices per group
GP_TARGET = 8000                     # target events on the gpsimd side

# ---- SWDGE (dma_gather) event share ----
SLOT_D = 6        # event slots per segment
SPD = 28          # segments per partition
C_EV = SPD * SLOT_D                  # 168 event columns per partition
NSEG = 128 * SPD                     # 3584 segments per core
SEG_OPS = 2       # seg-gather split into this many dma_gather ops
SEG_PER_OP = NSEG // SEG_OPS
EV_CHUNKS = 4     # v-side gather ops (one per SWDGE queue)
EV_CC = C_EV // EV_CHUNKS            # 42 event columns per chunk
EV_PER_CHUNK = 128 * EV_CC
GELEM = 64        # gather element size in f32 (256B rows)

F32 = mybir.dt.float32
F32R = mybir.dt.float32r
BF16 = mybir.dt.bfloat16
I16 = mybir.dt.int16
AF = mybir.ActivationFunctionType
OP = mybir.AluOpType

_CACHE: dict = {}
_DBG_SPLIT: list = []


def _tt(nc, out, in0, in1, op):
    return nc.vector.tensor_tensor(out, in0, in1, op=op)


def _build():
    if "nc" in _CACHE:
        return _CACHE["nc"]

    nc = bacc.Bacc(
        "TRN2", target_bir_lowering=False, debug=False, enable_asserts=False,
        num_swdge_queues=4,
    )

    # inputs coalesced into 3 blobs: per-DMA fixed cost (~2.4us) made 19
    # separate loads a ~46us critical-path prefix
    FB = 396          # f32 blob: zv(64) zvi(8) tb(10) t2b(10) ident(128)
    #                   ones16(8) ev_t(168)
    BB = 2 * N + EV_G                # bf16 blob: gtab (N x 2) | tmat
    IB = (SEG_OPS * SEG_PER_OP + EV_CHUNKS * EV_PER_CHUNK + NIDX) // 16
    zv_pad = nc.dram_tensor("zv_pad", [N, GELEM], F32, kind="ExternalInput").ap()
    fblob_d = nc.dram_tensor("fblob", [128, FB], F32, kind="ExternalInput").ap()
    bblob_d = nc.dram_tensor("bblob", [128, BB], BF16, kind="ExternalInput").ap()
    iblob_d = nc.dram_tensor("iblob", [128, IB], I16, kind="ExternalInput").ap()
    out_p = nc.dram_tensor("out_p", [128, 24], F32, kind="ExternalOutput").ap()

    with tile.TileContext(nc) as tc, ExitStack() as ctx:
        cpool = ctx.enter_context(tc.tile_pool(name="const", bufs=1))
        evpool = ctx.enter_context(tc.tile_pool(name="ev", bufs=1))

        # ---------------- input loads (3 coalesced blobs) ----------------
        ib_sb = evpool.tile([128, IB], I16)
        nc.sync.dma_start(ib_sb[:], iblob_d)
        fb_sb = cpool.tile([128, FB], F32)
        nc.sync.dma_start(fb_sb[:], fblob_d)
        bb_sb = evpool.tile([128, BB], BF16)
        nc.sync.dma_start(bb_sb[:], bblob_d)

        UW = SEG_PER_OP // 16
        VW = EV_PER_CHUNK // 16
        u_sb = ib_sb[:, 0:SEG_OPS * UW].rearrange("p (a b) -> p a b", a=SEG_OPS)
        v_sb = ib_sb[:, SEG_OPS * UW:SEG_OPS * UW + EV_CHUNKS * VW].rearrange(
            "p (a b) -> p a b", a=EV_CHUNKS
        )
        gidx = ib_sb[:, SEG_OPS * UW + EV_CHUNKS * VW:IB]
        zv_sb = fb_sb[:, 0:64].rearrange("p (c d) -> p c d", d=4)
        zvi_sb = fb_sb[:, 64:72].rearrange("p (c d) -> p c d", d=4)
        tb = fb_sb[:, 72:82]
        t2b = fb_sb[:, 82:92]
        ident = fb_sb[:, 92:220]
        ones16_f = fb_sb[:, 220:228]
        t_sb = fb_sb[:, 228:396]
        gtab = bb_sb[:, 0:2 * N].rearrange("p (n d) -> p n d", d=2)
        tmat = bb_sb[:, 2 * N:BB]

        acc = cpool.tile([128, 24], F32)
        nc.vector.memset(acc[:], 0.0)

        # ---------------- event gathers ----------------
        # gpsimd stream order matters: the SWDGE descriptor GENERATION
        # runs first (its DMA drain proceeds on the DMA engines while the
        # gpsimd engine moves on), then one library reload, then the
        # ap_gather for the gpsimd share.
        # SWDGE share: u-side one 256B row per SEGMENT; v-side one row
        # per event slot (pads gather v=u, t=0 -> exactly 0)
        d2all = evpool.tile([128, C_EV, 1], F32)
        seg = evpool.tile([128, SPD, GELEM], F32)
        for so in range(SEG_OPS):
            nc.gpsimd.dma_gather(
                seg[:, so * (SPD // SEG_OPS):(so + 1) * (SPD // SEG_OPS), :],
                zv_pad, u_sb[:, so, :], SEG_PER_OP, SEG_PER_OP, GELEM,
                single_packet=False, queue_num=so % 4,
            )
        dvg = ctx.enter_context(tc.tile_pool(name="dvg", bufs=4))
        b_tiles = []
        last_gather = None
        for ch in range(EV_CHUNKS):
            B = dvg.tile([128, EV_CC, GELEM], F32, tag="B", name="B")
            last_gather = nc.gpsimd.dma_gather(
                B[:], zv_pad, v_sb[:, ch, :], EV_PER_CHUNK, EV_PER_CHUNK, GELEM,
                single_packet=False, queue_num=ch % 4,
            )
            b_tiles.append(B)

        # gpsimd share: one ap_gather; table row n = [A_k(n), C_k(n)]
        # (d=2 bf16 = 4B per index); segment slots use component 0,
        # event slots component 1
        gout = evpool.tile([128, NIDX, 2], BF16)
        apg = nc.gpsimd.ap_gather(
            gout[:], gtab, gidx,
            channels=128, num_elems=N, d=2, num_idxs=NIDX,
        )
        # the SWDGE generation must run first: its drain proceeds on the
        # DMA engines while the slow ap_gather occupies the Q7 cores
        add_dep_helper(apg.ins, last_gather.ins, reason="gen before ap_gather")
        ones16 = evpool.tile([128, 8], BF16)
        nc.vector.tensor_copy(ones16[:], ones16_f)

        def emit_dma_event_math(ch, scratch_pool):
            B = b_tiles[ch]
            sc = EV_CC // SLOT_D                     # segments per chunk
            q0 = ch * sc
            shape4 = [128, sc, SLOT_D, 1]
            tse = (
                t_sb[:, ch * EV_CC:(ch + 1) * EV_CC]
                .rearrange("p (q j) -> p q j", j=SLOT_D)
                .unsqueeze(3)
            )

            def sv(d):  # seg channel d view broadcast over the slots
                return (
                    seg[:, q0:q0 + sc, d:d + 1]
                    .unsqueeze(2)
                    .to_broadcast(shape4)
                )

            def bv(d):  # B channel d view
                return B[:, :, d:d + 1].rearrange(
                    "p (q j) d -> p q j d", j=SLOT_D
                )

            dzx = scratch_pool.tile(shape4, F32, tag="w", name="dzx")
            dvx = scratch_pool.tile(shape4, F32, tag="w", name="dvx")
            dzy = scratch_pool.tile(shape4, F32, tag="w", name="dzy")
            dvy = scratch_pool.tile(shape4, F32, tag="w", name="dvy")
            _tt(nc, dzx[:], sv(0), bv(0), OP.subtract)
            _tt(nc, dvx[:], sv(2), bv(2), OP.subtract)
            _tt(nc, dvx[:], dvx[:], tse, OP.mult)
            _tt(nc, dzx[:], dzx[:], dvx[:], OP.add)          # dx
            _tt(nc, dzy[:], sv(1), bv(1), OP.subtract)
            _tt(nc, dvy[:], sv(3), bv(3), OP.subtract)
            _tt(nc, dvy[:], dvy[:], tse, OP.mult)
            _tt(nc, dzy[:], dzy[:], dvy[:], OP.add)          # dy
            _tt(nc, dzx[:], dzx[:], dzx[:], OP.mult)
            _tt(nc, dzy[:], dzy[:], dzy[:], OP.mult)
            d2v = d2all[:, ch * EV_CC:(ch + 1) * EV_CC, :].rearrange(
                "p (q j) d -> p q j d", j=SLOT_D
            )
            _tt(nc, d2v, dzx[:], dzy[:], OP.add)             # d^2

        # ---------------- j features  F[p, chunk, 0:8] ----------------
        # [1, a, b, c, zx, vx, zy, vy]; padded to 32 for the PE transpose
        F = cpool.tile([128, 16, 32], F32)
        zx = zv_sb[:, :, 0:1]
        zy = zv_sb[:, :, 1:2]
        vx = zv_sb[:, :, 2:3]
        vy = zv_sb[:, :, 3:4]
        s1 = cpool.tile([128, 16, 1], F32)
        nc.vector.memset(F[:, :, 0:1], 1.0)
        _tt(nc, F[:, :, 1:2], zx, zx, OP.mult)           # a = zx^2 + zy^2
        _tt(nc, s1[:], zy, zy, OP.mult)
        _tt(nc, F[:, :, 1:2], F[:, :, 1:2], s1[:], OP.add)
        s2 = cpool.tile([128, 16, 1], F32)
        _tt(nc, F[:, :, 2:3], zx, vx, OP.mult)           # b = 2(zx vx + zy vy)
        _tt(nc, s2[:], zy, vy, OP.mult)
        _tt(nc, F[:, :, 2:3], F[:, :, 2:3], s2[:], OP.add)
        nc.vector.tensor_scalar_mul(F[:, :, 2:3], F[:, :, 2:3], 2.0)
        s3 = cpool.tile([128, 16, 1], F32)
        _tt(nc, F[:, :, 3:4], vx, vx, OP.mult)           # c = vx^2 + vy^2
        _tt(nc, s3[:], vy, vy, OP.mult)
        _tt(nc, F[:, :, 3:4], F[:, :, 3:4], s3[:], OP.add)
        nc.vector.tensor_copy(F[:, :, 4:5], zx)
        nc.vector.tensor_copy(F[:, :, 5:6], vx)
        nc.vector.tensor_copy(F[:, :, 6:7], zy)
        nc.vector.tensor_copy(F[:, :, 7:8], vy)

        # ---------------- i features  L[p, it, s, 0:8] ----------------
        # [r, 1, t, t^2, -2x, -2tx, -2y, -2ty]
        L = cpool.tile([128, ITILES, S, 32], F32)
        izx = zvi_sb[:, :, 0:1]
        izy = zvi_sb[:, :, 1:2]
        ivx = zvi_sb[:, :, 2:3]
        ivy = zvi_sb[:, :, 3:4]
        ia = cpool.tile([128, ITILES, 1], F32)
        ib = cpool.tile([128, ITILES, 1], F32)
        ic = cpool.tile([128, ITILES, 1], F32)
        s4 = cpool.tile([128, ITILES, 1], F32)
        _tt(nc, ia[:], izx, izx, OP.mult)
        _tt(nc, s4[:], izy, izy, OP.mult)
        _tt(nc, ia[:], ia[:], s4[:], OP.add)
        s5 = cpool.tile([128, ITILES, 1], F32)
        _tt(nc, ib[:], izx, ivx, OP.mult)
        _tt(nc, s5[:], izy, ivy, OP.mult)
        _tt(nc, ib[:], ib[:], s5[:], OP.add)
        nc.vector.tensor_scalar_mul(ib[:], ib[:], 2.0)
        s6 = cpool.tile([128, ITILES, 1], F32)
        _tt(nc, ic[:], ivx, ivx, OP.mult)
        _tt(nc, s6[:], ivy, ivy, OP.mult)
        _tt(nc, ic[:], ic[:], s6[:], OP.add)

        def b_i(v):  # [128, ITILES, 1] -> [128, ITILES, S, 1]
            return v.unsqueeze(2).to_broadcast([128, ITILES, S, 1])

        tv = tb.unsqueeze(1).unsqueeze(3).to_broadcast([128, ITILES, S, 1])
        t2v = t2b.unsqueeze(1).unsqueeze(3).to_broadcast([128, ITILES, S, 1])

        nc.vector.memset(L[:, :, :, 1:2], 1.0)
        nc.vector.tensor_copy(L[:, :, :, 2:3], tv)
        nc.vector.tensor_copy(L[:, :, :, 3:4], t2v)
        Lx = cpool.tile([128, ITILES, S, 1], F32)
        _tt(nc, Lx[:], b_i(ivx), tv, OP.mult)            # x_i(s) = zx + vx t
        _tt(nc, Lx[:], Lx[:], b_i(izx), OP.add)
        nc.vector.tensor_scalar_mul(L[:, :, :, 4:5], Lx[:], -2.0)
        _tt(nc, L[:, :, :, 5:6], L[:, :, :, 4:5], tv, OP.mult)
        Ly = cpool.tile([128, ITILES, S, 1], F32)
        _tt(nc, Ly[:], b_i(ivy), tv, OP.mult)
        _tt(nc, Ly[:], Ly[:], b_i(izy), OP.add)
        nc.vector.tensor_scalar_mul(L[:, :, :, 6:7], Ly[:], -2.0)
        _tt(nc, L[:, :, :, 7:8], L[:, :, :, 6:7], tv, OP.mult)
        Lr = cpool.tile([128, ITILES, S, 1], F32)
        _tt(nc, L[:, :, :, 0:1], b_i(ib), tv, OP.mult)   # r = a + b t + c t^2
        _tt(nc, L[:, :, :, 0:1], L[:, :, :, 0:1], b_i(ia), OP.add)
        _tt(nc, Lr[:], b_i(ic), t2v, OP.mult)
        _tt(nc, L[:, :, :, 0:1], L[:, :, :, 0:1], Lr[:], OP.add)

        # ---------------- transposes (PE) ----------------
        T2 = cpool.tile([8, N], F32R)                    # G_j rows
        L2 = cpool.tile([8, ITILES * S, 128], F32R)      # F_i(s) rows
        with tc.tile_pool(name="tp", bufs=4, space="PSUM") as tpp:
            for c in range(16):
                pt = tpp.tile([32, 128], F32, tag="pt", name="pt")
                nc.tensor.transpose(pt[:], F[:, c, :], ident)
                nc.vector.tensor_copy(T2[:, c * 128:(c + 1) * 128], pt[0:8, :])
            for it in range(ITILES):
                for s in range(S):
                    pt = tpp.tile([32, 128], F32, tag="pt", name="pt")
                    nc.tensor.transpose(pt[:], L[:, it, s, :], ident)
                    nc.vector.tensor_copy(L2[:, it * S + s, :], pt[0:8, :])

        # gpsimd-share P = A * C * T views
        P = evpool.tile([128, SEG_G, SLOT_G, 1], BF16)
        shape4g = [128, SEG_G, SLOT_G, 1]
        a_view = gout[:, 0:SEG_G, 0:1].unsqueeze(2).to_broadcast(shape4g)
        c_view = gout[:, SEG_G:NIDX, 1:2].rearrange(
            "p (q j) d -> p q j d", j=SLOT_G
        )
        t_view = tmat.rearrange("p (q j) -> p q j", j=SLOT_G).unsqueeze(3)
        d_ev = evpool.tile([128, C_EV, 1], F32)

        # ---------------- main pairwise loop ----------------
        sq_insts = [[] for _ in range(ITILES)]
        ex_insts = [[] for _ in range(ITILES)]
        with tc.tile_pool(name="qp", bufs=2, space="PSUM") as qpool, \
                tc.tile_pool(name="wp", bufs=12) as wpool:
            for it in range(ITILES):
                for s in range(S):
                    q = qpool.tile([128, N], F32, tag="q", name="q")
                    for kk in range(4):
                        nc.tensor.matmul(
                            q[:, kk * 512:(kk + 1) * 512],
                            L2[:, it * S + s, :],
                            T2[:, kk * 512:(kk + 1) * 512],
                            start=True, stop=True,
                        )
                    w = wpool.tile([128, N], BF16, tag="w", name="w")
                    nc.vector.tensor_scalar_max(w[:], q[:], 0.0)
                    col = it * S + s
                    sq = nc.scalar.activation(w[:], w[:], AF.Sqrt)
                    ex = nc.scalar.activation(
                        w[:], w[:], AF.Exp, scale=-1.0,
                        accum_out=acc[:, col:col + 1],
                    )
                    sq_insts[it].append(sq)
                    ex_insts[it].append(ex)

            # ---- event tails, at the END of every engine stream ----
            # gpsimd share: DVE product, PE channel reduce (single PSUM
            # tile, PSUM->SBUF relu copies between rounds), ACT sqrt
            _tt(nc, P[:], a_view, c_view, OP.mult)
            _tt(nc, P[:], P[:], t_view, OP.mult)
            q_ev = qpool.tile([128, N], F32, tag="q", name="q")
            ev_d2 = evpool.tile([8, EV_G], F32)
            pm = P[:].rearrange("p q j d -> p (q j d)")
            nmm = (EV_G + 511) // 512
            for r in range(nmm):
                c0 = (r % 4) * 512
                cw = min(512, EV_G - 512 * r)
                nc.tensor.matmul(
                    q_ev[0:8, c0:c0 + cw],
                    ones16[:],
                    pm[:, 512 * r:512 * r + cw],
                    start=True, stop=True,
                )
                nc.vector.tensor_scalar_max(
                    ev_d2[:, 512 * r:512 * r + cw],
                    q_ev[0:8, c0:c0 + cw],
                    0.0,
                )
            w_ev = evpool.tile([8, EV_G], BF16)
            ev_sq_g = nc.scalar.activation(
                w_ev[:], ev_d2[:], AF.Sqrt,
                accum_out=acc[0:8, 20:21],
            )

            # SWDGE share: distance algebra per chunk, then one sqrt
            for ch in range(EV_CHUNKS):
                emit_dma_event_math(ch, wpool)
            ev_sq_d = nc.scalar.activation(
                d_ev[:], d2all[:], AF.Sqrt, accum_out=acc[:, 21:22]
            )

            # ACT phase order: sqrt(i0) exp(i0) sqrt(i1) exp(i1) ev_g ev_d.
            # The event sqrts land last: their PE/DVE inputs are only
            # ready near the end of the main loop, and must not gate the
            # exp phases.
            order = (
                sq_insts[0] + ex_insts[0] + sq_insts[1] + ex_insts[1]
                + [ev_sq_g, ev_sq_d]
            )
            for a, b in zip(order[1:], order[:-1]):
                add_dep_helper(a.ins, b.ins, reason="act table phase order")

            nc.sync.dma_start(out_p, acc[:])

    nc.compile()
    _CACHE["nc"] = nc
    return nc


# trilinear channels: (A_k(u), B_k(t) power, C_k(v)); a = zx^2+zy^2,
# b = 2(zx vx + zy vy), c = vx^2+vy^2
# feature columns: [1, a, b, c, zx, zy, vx, vy]
_ACH = [1, 0, 4, 5, 2, 0, 4, 6, 5, 7, 3, 0, 6, 7]   # A feature index
_ASC = [1., 1., -2., -2., 1., 1., -2., -2., -2., -2., 1., 1., -2., -2.]
_BPOW = [0, 0, 0, 0, 1, 1, 1, 1, 1, 1, 2, 2, 2, 2]  # power of t
_CCH = [0, 1, 4, 5, 0, 2, 6, 4, 7, 5, 0, 3, 6, 7]   # C feature index


def _node_features(zv):
    zx, zy, vx, vy = zv[:, 0], zv[:, 1], zv[:, 2], zv[:, 3]
    a = zx * zx + zy * zy
    b = 2.0 * (zx * vx + zy * vy)
    c = vx * vx + vy * vy
    one = np.ones_like(a)
    return np.stack([one, a, b, c, zx, zy, vx, vy], axis=1)  # [N, 8]


def _marshal(inputs):
    import ml_dtypes

    z0 = np.asarray(inputs["z0"], dtype=np.float32)
    v0 = np.asarray(inputs["v0"], dtype=np.float32)
    uv = np.asarray(inputs["data_uv"], dtype=np.int32)
    tt = np.asarray(inputs["data_t"], dtype=np.float32)
    t0 = np.float32(np.asarray(inputs["t0"]).reshape(-1)[0])
    tn = np.float32(np.asarray(inputs["tn"]).reshape(-1)[0])

    zv = np.ascontiguousarray(np.concatenate([z0, v0], axis=1)).astype(np.float32)
    dt = np.float32((tn - t0) / np.float32(S))
    tmid = (t0 + (np.arange(S, dtype=np.float32) + np.float32(0.5)) * dt).astype(
        np.float32
    )
    tb = np.ascontiguousarray(np.broadcast_to(tmid, (128, S))).astype(np.float32)
    t2b = (tb * tb).astype(np.float32)

    zv_pad = np.zeros((N, GELEM), np.float32)
    zv_pad[:, 0:4] = zv

    feats = _node_features(zv.astype(np.float64)).astype(np.float32)  # [N, 8]
    gtab = np.zeros((128, N, 2), np.float32)
    for k in range(14):
        for g in range(NG):
            gtab[16 * g + k, :, 0] = _ASC[k] * feats[:, _ACH[k]]
            gtab[16 * g + k, :, 1] = feats[:, _CCH[k]]
    gtab = gtab.astype(ml_dtypes.bfloat16)

    E = uv.shape[0]
    assert E <= NCORES * EV_PER_CORE
    u_all = uv[:, 0].astype(np.int64)
    v_all = uv[:, 1].astype(np.int64)

    def split_core(u, v, t):
        """Assign each u-node's events wholly to the gpsimd or the SWDGE
        share; fill gpsimd groups (balanced) up to GP_TARGET events."""
        counts = np.bincount(u, minlength=N)
        order = np.argsort(-counts, kind="stable")
        g_ev = np.zeros(NG, np.int64)
        g_seg = np.zeros(NG, np.int64)
        node_g = np.full(N, -1, np.int64)   # -1 -> SWDGE share
        total = 0
        for n in order:
            c = int(counts[n])
            if c == 0 or total >= GP_TARGET:
                continue
            segs = -(-c // SLOT_G)
            g = int(np.argmin(g_ev))
            if g_ev[g] + segs * SLOT_G > EV_G or g_seg[g] + segs > SEG_G:
                continue
            node_g[n] = g
            g_ev[g] += segs * SLOT_G        # reserve whole segments
            g_seg[g] += segs
            total += c
        return node_g

    def pack_gp(u, v, t, node_g):
        """gpsimd share: wrapped gather index list + T tensor."""
        sel = node_g[u] >= 0
        us, vs, ts = u[sel], v[sel], t[sel]
        gs = node_g[us]
        order = np.argsort(us, kind="stable")
        us, vs, ts, gs = us[order], vs[order], ts[order], gs[order]

        seg_u = np.zeros((NG, SEG_G), np.int64)
        ev_vv = np.zeros((NG, SEG_G, SLOT_G), np.int64)
        ev_tt = np.zeros((NG, SEG_G, SLOT_G), np.float32)
        ev_mm = np.zeros((NG, SEG_G, SLOT_G), np.float32)
        seg_cnt = np.zeros(NG, np.int64)
        i = 0
        while i < len(us):
            j = i
            while j < len(us) and us[j] == us[i]:
                j += 1
            g = int(gs[i])
            for s0 in range(i, j, SLOT_G):
                q = seg_cnt[g]
                assert q < SEG_G, "gp segment overflow"
                seg_cnt[g] += 1
                e0 = min(s0 + SLOT_G, j)
                seg_u[g, q] = us[i]
                ev_vv[g, q, : e0 - s0] = vs[s0:e0]
                ev_tt[g, q, : e0 - s0] = ts[s0:e0]
                ev_mm[g, q, : e0 - s0] = 1.0
            i = j
        idx_flat = np.concatenate(
            [seg_u, ev_vv.reshape(NG, EV_G)], axis=1
        ).astype(np.int16)
        gidx = np.zeros((128, NIDX // 16), np.int16)
        for g in range(NG):
            gidx[16 * g:16 * (g + 1), :] = (
                idx_flat[g].reshape(NIDX // 16, 16).T
            )
        tmat = np.zeros((128, EV_G), np.float32)
        tflat = ev_tt.reshape(NG, EV_G)
        mflat = ev_mm.reshape(NG, EV_G)
        for k in range(14):
            p = _BPOW[k]
            for g in range(NG):
                tmat[16 * g + k, :] = (tflat[g] ** p) * mflat[g]
        return gidx, tmat.astype(ml_dtypes.bfloat16)

    def pack_dma(u, v, t, node_g):
        """SWDGE share: baseline segment layout (pads v=u, t=0)."""
        sel = node_g[u] < 0
        us, vs, ts = u[sel], v[sel], t[sel]
        order = np.argsort(us, kind="stable")
        us, vs, ts = us[order], vs[order], ts[order]
        starts = np.flatnonzero(np.r_[True, us[1:] != us[:-1]])
        ends = np.r_[starts[1:], len(us)]
        seg_nodes = np.zeros((128, SPD), np.int16)
        v_slots = np.zeros((128, SPD, SLOT_D), np.int16)
        t_slots = np.zeros((128, SPD, SLOT_D), np.float32)
        counts = np.zeros(128, np.int64)
        i = 0
        for s0, e0 in zip(starts, ends):
            n = us[s0]
            for j in range(s0, e0, SLOT_D):
                p = i % 128
                q = counts[p]
                counts[p] += 1
                assert q < SPD, "dma segment overflow; raise SPD"
                i += 1
                seg_nodes[p, q] = n
                va = vs[j:min(j + SLOT_D, e0)]
                ta = ts[j:min(j + SLOT_D, e0)]
                v_slots[p, q, :] = n
                v_slots[p, q, : len(va)] = va
                t_slots[p, q, : len(ta)] = ta
        return (
            seg_nodes,
            v_slots.reshape(128, C_EV),
            t_slots.reshape(128, C_EV),
        )

    def wrap16(x, nops, per_op):
        w = x.reshape(nops, per_op // 16, 16).transpose(2, 0, 1)
        return np.ascontiguousarray(np.tile(w, (8, 1, 1)))

    ones16 = np.zeros((128, 8), np.float32)
    for g in range(NG):
        ones16[16 * g:16 * (g + 1), g] = 1.0

    ident_np = np.eye(128, dtype=np.float32)
    in_maps = []
    _DBG_SPLIT.clear()
    for k in range(NCORES):
        sl = slice(k * EV_PER_CORE, (k + 1) * EV_PER_CORE)
        u, v, t = u_all[sl], v_all[sl], tt[sl]
        node_g = split_core(u, v, t)
        zv64 = zv.astype(np.float64)

        def _dsum(mask):
            uu, vv, tt_ = u[mask], v[mask], t[mask]
            dx = (zv64[uu, 0] - zv64[vv, 0]) + (zv64[uu, 2] - zv64[vv, 2]) * tt_
            dy = (zv64[uu, 1] - zv64[vv, 1]) + (zv64[uu, 3] - zv64[vv, 3]) * tt_
            return float(np.sqrt(dx * dx + dy * dy).sum())

        _DBG_SPLIT.append(
            (_dsum(node_g[u] >= 0), _dsum(node_g[u] < 0), int((node_g[u] >= 0).sum()))
        )
        gidx, tmat = pack_gp(u, v, t, node_g)
        seg_nodes, v_slots, t_slots = pack_dma(u, v, t, node_g)
        seg_list = seg_nodes.T.reshape(-1)
        v_list = (
            v_slots.reshape(128, EV_CHUNKS, EV_CC)
            .transpose(1, 2, 0)
            .reshape(-1)
        )
        zvi = zv[k * 256:(k + 1) * 256]
        fblob = np.concatenate(
            [
                zv.reshape(16, 128, 4).transpose(1, 0, 2).reshape(128, 64),
                zvi.reshape(2, 128, 4).transpose(1, 0, 2).reshape(128, 8),
                tb,
                t2b,
                ident_np,
                ones16,
                t_slots,
            ],
            axis=1,
        ).astype(np.float32)
        bblob = np.concatenate(
            [gtab.reshape(128, 2 * N), tmat], axis=1
        )
        iblob = np.concatenate(
            [
                wrap16(seg_list, SEG_OPS, SEG_PER_OP).reshape(128, -1),
                wrap16(v_list, EV_CHUNKS, EV_PER_CHUNK).reshape(128, -1),
                gidx,
            ],
            axis=1,
        ).astype(np.int16)
        in_maps.append(
            {
                "zv_pad": zv_pad,
                "fblob": np.ascontiguousarray(fblob),
                "bblob": np.ascontiguousarray(bblob),
                "iblob": np.ascontiguousarray(iblob),
            }
        )
    return in_maps, (float(t0), float(tn), E)


def _np_event_total(inputs, core):
    """float64 reference event-distance sum for one core's slice."""
    z0 = np.asarray(inputs["z0"], np.float64)
    v0 = np.asarray(inputs["v0"], np.float64)
    uv = np.asarray(inputs["data_uv"], np.int64)
    tt = np.asarray(inputs["data_t"], np.float64)
    sl = slice(core * EV_PER_CORE, (core + 1) * EV_PER_CORE)
    u, v, t = uv[sl, 0], uv[sl, 1], tt[sl]
    dx = (z0[u, 0] - z0[v, 0]) + (v0[u, 0] - v0[v, 0]) * t
    dy = (z0[u, 1] - z0[v, 1]) + (v0[u, 1] - v0[v, 1]) * t
    return np.sqrt(dx * dx + dy * dy).sum()


def _combine(core_outs, beta, t0, tn, E):
    """core_outs: list of [128, 24] float32 partial-sum tensors."""
    exp_sum = 0.0
    ev_sum = 0.0
    for o in core_outs:
        o = np.asarray(o, dtype=np.float64)
        exp_sum += o[:, 0 : ITILES * S].sum()
        ev_sum += o[:, 20].sum() + o[:, 21].sum()
    b = float(beta)
    dt = (tn - t0) / S
    event_intensity = E * b - ev_sum
    non_event = np.exp(b) * (exp_sum - S * N) / 2.0 * dt
    return np.float32(event_intensity - 1.0 * non_event)


def kernel(**inputs) -> np.ndarray:
    from concourse.bass_utils import run_bass_kernel_spmd

    nc = _build()
    in_maps, (t0, tn, E) = _marshal(inputs)
    res = run_bass_kernel_spmd(nc, in_maps, core_ids=list(range(NCORES)))
    beta = float(np.asarray(inputs["beta"]).reshape(-1)[0])
    out = _combine([r["out_p"] for r in res.results], beta, t0, tn, E)
    return np.asarray(out, dtype=np.float32)


# revision 32
# speedup vs baseline: 1.0895x; 1.0895x over previous
"""Trainium2 Bass kernel for BasicEuclideanDistModel (gnn_message_passing).

Math:
  result = sum_e (beta - ||dz_e + dv_e t_e||)
           - dt * sum_{i<j, s} exp(beta - ||z_i(t_s) - z_j(t_s)||)

Device strategy (8 cores, data parallel):
  * Non-event term: full NxN pairwise distances (halved on host).
    d^2(i,j,s) = F_i(s) . G_j  (K=8 inner product, G time-independent).
    One [8,128]x[8,2048] matmul (fp32r) per (i-tile, s) computes the
    d^2 supertile; DVE relu clamps rounding negatives, ACT computes
    sqrt then exp(-d) with fused row sums.  Each core owns 2 of the
    16 i-tiles, all j, all 10 samples.
  * Event term, split across two independent engines working in
    parallel (events of one u-node always stay together):
    - gpsimd share: d^2(u,v,t) = sum_k A_k(u) B_k(t) C_k(v), a
      14-channel trilinear decomposition with B_k in {1,t,t^2}.
      Events form 8 groups (one per Q7 tile); partition 16g+k holds
      channel k.  ONE ap_gather (SBUF gather, ~27.5ns/idx/core,
      shared index list per group) fetches A_k per segment and C_k
      per event from a [128, N, 2] bf16 channel table; DVE forms
      P = A*C*T (host T = B_k(t)*mask), PE reduces channels with a
      block-ones stationary, one ACT sqrt row-sum -> acc col 20.
    - SWDGE share: baseline scheme -- events grouped by u into
      segments laid out [128, SPD, SLOTD]; dma_gather fetches 256B
      rows (u per segment, v per slot, ~3.8ns/desc aggregate); DVE
      distance algebra, ACT sqrt row-sum -> acc col 21.  Pad slots
      use v=u, t=0 (exactly 0 contribution).
  * beta enters only as a scalar factor / offset -> folded in on host.
  Host combines 8 cores' [128, 24] partial-sum tensors (pure unshard/
  reduction of partials).
"""

import os
import numpy as np


def _import_concourse():
    try:
        import concourse  # noqa: F401
    except ImportError:
        import sys

        for p in ("/opt/trn_rl_repo", "/root/.axon_site/_ro/trn_rl_repo"):
            if os.path.isdir(p) and p not in sys.path:
                sys.path.insert(0, p)


_import_concourse()

from contextlib import ExitStack  # noqa: E402

import concourse.bacc as bacc  # noqa: E402
import concourse.mybir as mybir  # noqa: E402
import concourse.tile as tile  # noqa: E402
from concourse.tile_rust import add_dep_helper  # noqa: E402

N = 2048          # nodes
S = 10            # Riemann samples
NCORES = 8
ITILES = 2        # 128-row i-tiles per core
EV_PER_CORE = 200000 // NCORES       # real events per core

# ---- gpsimd (ap_gather) event share: the highest-count u-nodes ----
NG = 8            # groups (one per Q7 tile of 16 partitions)
SLOT_G = 6        # event slots per segment
SEG_G = 208       # segments per group
EV_G = SEG_G * SLOT_G                # 1248 event slots per group
NIDX = SEG_G + EV_G                  # 1456 gather indices per group
GP_TARGET = 8000                     # target events on the gpsimd side

# ---- SWDGE (dma_gather) event share ----
SLOT_D = 6        # event slots per segment
SPD = 28          # segments per partition
C_EV = SPD * SLOT_D                  # 168 event columns per partition
NSEG = 128 * SPD                     # 3584 segments per core
SEG_OPS = 2       # seg-gather split into this many dma_gather ops
SEG_PER_OP = NSEG // SEG_OPS
EV_CHUNKS = 4     # v-side gather ops (one per SWDGE queue)
EV_CC = C_EV // EV_CHUNKS            # 42 event columns per chunk
EV_PER_CHUNK = 128 * EV_CC
GELEM = 64        # gather element size in f32 (256B rows)

F32 = mybir.dt.float32
F32R = mybir.dt.float32r
BF16 = mybir.dt.bfloat16
I16 = mybir.dt.int16
AF = mybir.ActivationFunctionType
OP = mybir.AluOpType

_CACHE: dict = {}
_DBG_SPLIT: list = []


def _tt(nc, out, in0, in1, op):
    return nc.vector.tensor_tensor(out, in0, in1, op=op)


def _build():
    if "nc" in _CACHE:
        return _CACHE["nc"]

    nc = bacc.Bacc(
        "TRN2", target_bir_lowering=False, debug=False, enable_asserts=False,
        num_swdge_queues=4,
    )

    # inputs coalesced into 3 blobs: per-DMA fixed cost (~2.4us) made 19
    # separate loads a ~46us critical-path prefix
    FB = 396          # f32 blob: zv(64) zvi(8) tb(10) t2b(10) ident(128)
    #                   ones16(8) ev_t(168)
    BB = 2 * N + EV_G                # bf16 blob: gtab (N x 2) | tmat
    IB = (SEG_OPS * SEG_PER_OP + EV_CHUNKS * EV_PER_CHUNK + NIDX) // 16
    zv_pad = nc.dram_tensor("zv_pad", [N, GELEM], F32, kind="ExternalInput").ap()
    fblob_d = nc.dram_tensor("fblob", [128, FB], F32, kind="ExternalInput").ap()
    bblob_d = nc.dram_tensor("bblob", [128, BB], BF16, kind="ExternalInput").ap()
    iblob_d = nc.dram_tensor("iblob", [128, IB], I16, kind="ExternalInput").ap()
    out_p = nc.dram_tensor("out_p", [128, 24], F32, kind="ExternalOutput").ap()

    with tile.TileContext(nc) as tc, ExitStack() as ctx:
        cpool = ctx.enter_context(tc.tile_pool(name="const", bufs=1))
        evpool = ctx.enter_context(tc.tile_pool(name="ev", bufs=1))

        # ---------------- input loads (3 coalesced blobs) ----------------
        ib_sb = evpool.tile([128, IB], I16)
        nc.sync.dma_start(ib_sb[:], iblob_d)
        fb_sb = cpool.tile([128, FB], F32)
        nc.sync.dma_start(fb_sb[:], fblob_d)
        bb_sb = evpool.tile([128, BB], BF16)
        nc.sync.dma_start(bb_sb[:], bblob_d)

        UW = SEG_PER_OP // 16
        VW = EV_PER_CHUNK // 16
        u_sb = ib_sb[:, 0:SEG_OPS * UW].rearrange("p (a b) -> p a b", a=SEG_OPS)
        v_sb = ib_sb[:, SEG_OPS * UW:SEG_OPS * UW + EV_CHUNKS * VW].rearrange(
            "p (a b) -> p a b", a=EV_CHUNKS
        )
        gidx = ib_sb[:, SEG_OPS * UW + EV_CHUNKS * VW:IB]
        zv_sb = fb_sb[:, 0:64].rearrange("p (c d) -> p c d", d=4)
        zvi_sb = fb_sb[:, 64:72].rearrange("p (c d) -> p c d", d=4)
        tb = fb_sb[:, 72:82]
        t2b = fb_sb[:, 82:92]
        ident = fb_sb[:, 92:220]
        ones16_f = fb_sb[:, 220:228]
        t_sb = fb_sb[:, 228:396]
        gtab = bb_sb[:, 0:2 * N].rearrange("p (n d) -> p n d", d=2)
        tmat = bb_sb[:, 2 * N:BB]

        acc = cpool.tile([128, 24], F32)
        nc.vector.memset(acc[:], 0.0)

        # ---------------- event gathers ----------------
        # gpsimd stream order matters: the SWDGE descriptor GENERATION
        # runs first (its DMA drain proceeds on the DMA engines while the
        # gpsimd engine moves on), then one library reload, then the
        # ap_gather for the gpsimd share.
        # SWDGE share: u-side one 256B row per SEGMENT; v-side one row
        # per event slot (pads gather v=u, t=0 -> exactly 0)
        d2all = evpool.tile([128, C_EV, 1], F32)
        seg = evpool.tile([128, SPD, GELEM], F32)
        for so in range(SEG_OPS):
            nc.gpsimd.dma_gather(
                seg[:, so * (SPD // SEG_OPS):(so + 1) * (SPD // SEG_OPS), :],
                zv_pad, u_sb[:, so, :], SEG_PER_OP, SEG_PER_OP, GELEM,
                single_packet=False, queue_num=so % 4,
            )
        dvg = ctx.enter_context(tc.tile_pool(name="dvg", bufs=4))
        b_tiles = []
        last_gather = None
        for ch in range(EV_CHUNKS):
            B = dvg.tile([128, EV_CC, GELEM], F32, tag="B", name="B")
            last_gather = nc.gpsimd.dma_gather(
                B[:], zv_pad, v_sb[:, ch, :], EV_PER_CHUNK, EV_PER_CHUNK, GELEM,
                single_packet=False, queue_num=ch % 4,
            )
            b_tiles.append(B)

        # gpsimd share: one ap_gather; table row n = [A_k(n), C_k(n)]
        # (d=2 bf16 = 4B per index); segment slots use component 0,
        # event slots component 1
        gout = evpool.tile([128, NIDX, 2], BF16)
        apg = nc.gpsimd.ap_gather(
            gout[:], gtab, gidx,
            channels=128, num_elems=N, d=2, num_idxs=NIDX,
        )
        _ = (apg, last_gather)  # scheduler orders the gpsimd stream itself
        ones16 = evpool.tile([128, 8], BF16)
        nc.vector.tensor_copy(ones16[:], ones16_f)

        def emit_dma_event_math(ch, scratch_pool):
            B = b_tiles[ch]
            sc = EV_CC // SLOT_D                     # segments per chunk
            q0 = ch * sc
            shape4 = [128, sc, SLOT_D, 1]
            tse = (
                t_sb[:, ch * EV_CC:(ch + 1) * EV_CC]
                .rearrange("p (q j) -> p q j", j=SLOT_D)
                .unsqueeze(3)
            )

            def sv(d):  # seg channel d view broadcast over the slots
                return (
                    seg[:, q0:q0 + sc, d:d + 1]
                    .unsqueeze(2)
                    .to_broadcast(shape4)
                )

            def bv(d):  # B channel d view
                return B[:, :, d:d + 1].rearrange(
                    "p (q j) d -> p q j d", j=SLOT_D
                )

            dzx = scratch_pool.tile(shape4, F32, tag="w", name="dzx")
            dvx = scratch_pool.tile(shape4, F32, tag="w", name="dvx")
            dzy = scratch_pool.tile(shape4, F32, tag="w", name="dzy")
            dvy = scratch_pool.tile(shape4, F32, tag="w", name="dvy")
            _tt(nc, dzx[:], sv(0), bv(0), OP.subtract)
            _tt(nc, dvx[:], sv(2), bv(2), OP.subtract)
            _tt(nc, dvx[:], dvx[:], tse, OP.mult)
            _tt(nc, dzx[:], dzx[:], dvx[:], OP.add)          # dx
            _tt(nc, dzy[:], sv(1), bv(1), OP.subtract)
            _tt(nc, dvy[:], sv(3), bv(3), OP.subtract)
            _tt(nc, dvy[:], dvy[:], tse, OP.mult)
            _tt(nc, dzy[:], dzy[:], dvy[:], OP.add)          # dy
            _tt(nc, dzx[:], dzx[:], dzx[:], OP.mult)
            _tt(nc, dzy[:], dzy[:], dzy[:], OP.mult)
            d2v = d2all[:, ch * EV_CC:(ch + 1) * EV_CC, :].rearrange(
                "p (q j) d -> p q j d", j=SLOT_D
            )
            _tt(nc, d2v, dzx[:], dzy[:], OP.add)             # d^2

        # ---------------- j features  F[p, chunk, 0:8] ----------------
        # [1, a, b, c, zx, vx, zy, vy]; padded to 32 for the PE transpose
        F = cpool.tile([128, 16, 32], F32)
        zx = zv_sb[:, :, 0:1]
        zy = zv_sb[:, :, 1:2]
        vx = zv_sb[:, :, 2:3]
        vy = zv_sb[:, :, 3:4]
        s1 = cpool.tile([128, 16, 1], F32)
        nc.vector.memset(F[:, :, 0:1], 1.0)
        _tt(nc, F[:, :, 1:2], zx, zx, OP.mult)           # a = zx^2 + zy^2
        _tt(nc, s1[:], zy, zy, OP.mult)
        _tt(nc, F[:, :, 1:2], F[:, :, 1:2], s1[:], OP.add)
        s2 = cpool.tile([128, 16, 1], F32)
        _tt(nc, F[:, :, 2:3], zx, vx, OP.mult)           # b = 2(zx vx + zy vy)
        _tt(nc, s2[:], zy, vy, OP.mult)
        _tt(nc, F[:, :, 2:3], F[:, :, 2:3], s2[:], OP.add)
        nc.vector.tensor_scalar_mul(F[:, :, 2:3], F[:, :, 2:3], 2.0)
        s3 = cpool.tile([128, 16, 1], F32)
        _tt(nc, F[:, :, 3:4], vx, vx, OP.mult)           # c = vx^2 + vy^2
        _tt(nc, s3[:], vy, vy, OP.mult)
        _tt(nc, F[:, :, 3:4], F[:, :, 3:4], s3[:], OP.add)
        nc.vector.tensor_copy(F[:, :, 4:5], zx)
        nc.vector.tensor_copy(F[:, :, 5:6], vx)
        nc.vector.tensor_copy(F[:, :, 6:7], zy)
        nc.vector.tensor_copy(F[:, :, 7:8], vy)

        # ---------------- i features  L[p, it, s, 0:8] ----------------
        # [r, 1, t, t^2, -2x, -2tx, -2y, -2ty]
        L = cpool.tile([128, ITILES, S, 32], F32)
        izx = zvi_sb[:, :, 0:1]
        izy = zvi_sb[:, :, 1:2]
        ivx = zvi_sb[:, :, 2:3]
        ivy = zvi_sb[:, :, 3:4]
        ia = cpool.tile([128, ITILES, 1], F32)
        ib = cpool.tile([128, ITILES, 1], F32)
        ic = cpool.tile([128, ITILES, 1], F32)
        s4 = cpool.tile([128, ITILES, 1], F32)
        _tt(nc, ia[:], izx, izx, OP.mult)
        _tt(nc, s4[:], izy, izy, OP.mult)
        _tt(nc, ia[:], ia[:], s4[:], OP.add)
        s5 = cpool.tile([128, ITILES, 1], F32)
        _tt(nc, ib[:], izx, ivx, OP.mult)
        _tt(nc, s5[:], izy, ivy, OP.mult)
        _tt(nc, ib[:], ib[:], s5[:], OP.add)
        nc.vector.tensor_scalar_mul(ib[:], ib[:], 2.0)
        s6 = cpool.tile([128, ITILES, 1], F32)
        _tt(nc, ic[:], ivx, ivx, OP.mult)
        _tt(nc, s6[:], ivy, ivy, OP.mult)
        _tt(nc, ic[:], ic[:], s6[:], OP.add)

        def b_i(v):  # [128, ITILES, 1] -> [128, ITILES, S, 1]
            return v.unsqueeze(2).to_broadcast([128, ITILES, S, 1])

        tv = tb.unsqueeze(1).unsqueeze(3).to_broadcast([128, ITILES, S, 1])
        t2v = t2b.unsqueeze(1).unsqueeze(3).to_broadcast([128, ITILES, S, 1])

        nc.vector.memset(L[:, :, :, 1:2], 1.0)
        nc.vector.tensor_copy(L[:, :, :, 2:3], tv)
        nc.vector.tensor_copy(L[:, :, :, 3:4], t2v)
        Lx = cpool.tile([128, ITILES, S, 1], F32)
        _tt(nc, Lx[:], b_i(ivx), tv, OP.mult)            # x_i(s) = zx + vx t
        _tt(nc, Lx[:], Lx[:], b_i(izx), OP.add)
        nc.vector.tensor_scalar_mul(L[:, :, :, 4:5], Lx[:], -2.0)
        _tt(nc, L[:, :, :, 5:6], L[:, :, :, 4:5], tv, OP.mult)
        Ly = cpool.tile([128, ITILES, S, 1], F32)
        _tt(nc, Ly[:], b_i(ivy), tv, OP.mult)
        _tt(nc, Ly[:], Ly[:], b_i(izy), OP.add)
        nc.vector.tensor_scalar_mul(L[:, :, :, 6:7], Ly[:], -2.0)
        _tt(nc, L[:, :, :, 7:8], L[:, :, :, 6:7], tv, OP.mult)
        Lr = cpool.tile([128, ITILES, S, 1], F32)
        _tt(nc, L[:, :, :, 0:1], b_i(ib), tv, OP.mult)   # r = a + b t + c t^2
        _tt(nc, L[:, :, :, 0:1], L[:, :, :, 0:1], b_i(ia), OP.add)
        _tt(nc, Lr[:], b_i(ic), t2v, OP.mult)
        _tt(nc, L[:, :, :, 0:1], L[:, :, :, 0:1], Lr[:], OP.add)

        # ---------------- transposes (PE) ----------------
        T2 = cpool.tile([8, N], F32R)                    # G_j rows
        L2 = cpool.tile([8, ITILES * S, 128], F32R)      # F_i(s) rows
        with tc.tile_pool(name="tp", bufs=4, space="PSUM") as tpp:
            for c in range(16):
                pt = tpp.tile([32, 128], F32, tag="pt", name="pt")
                nc.tensor.transpose(pt[:], F[:, c, :], ident)
                nc.vector.tensor_copy(T2[:, c * 128:(c + 1) * 128], pt[0:8, :])
            for it in range(ITILES):
                for s in range(S):
                    pt = tpp.tile([32, 128], F32, tag="pt", name="pt")
                    nc.tensor.transpose(pt[:], L[:, it, s, :], ident)
                    nc.vector.tensor_copy(L2[:, it * S + s, :], pt[0:8, :])

        # gpsimd-share P = A * C * T views
        P = evpool.tile([128, SEG_G, SLOT_G, 1], BF16)
        shape4g = [128, SEG_G, SLOT_G, 1]
        a_view = gout[:, 0:SEG_G, 0:1].unsqueeze(2).to_broadcast(shape4g)
        c_view = gout[:, SEG_G:NIDX, 1:2].rearrange(
            "p (q j) d -> p q j d", j=SLOT_G
        )
        t_view = tmat.rearrange("p (q j) -> p q j", j=SLOT_G).unsqueeze(3)
        d_ev = evpool.tile([128, C_EV, 1], F32)

        # ---------------- main pairwise loop ----------------
        sq_insts = [[] for _ in range(ITILES)]
        ex_insts = [[] for _ in range(ITILES)]
        with tc.tile_pool(name="qp", bufs=2, space="PSUM") as qpool, \
                tc.tile_pool(name="wp", bufs=12) as wpool:
            for it in range(ITILES):
                for s in range(S):
                    q = qpool.tile([128, N], F32, tag="q", name="q")
                    for kk in range(4):
                        nc.tensor.matmul(
                            q[:, kk * 512:(kk + 1) * 512],
                            L2[:, it * S + s, :],
                            T2[:, kk * 512:(kk + 1) * 512],
                            start=True, stop=True,
                        )
                    w = wpool.tile([128, N], BF16, tag="w", name="w")
                    nc.vector.tensor_scalar_max(w[:], q[:], 0.0)
                    col = it * S + s
                    sq = nc.scalar.activation(w[:], w[:], AF.Sqrt)
                    ex = nc.scalar.activation(
                        w[:], w[:], AF.Exp, scale=-1.0,
                        accum_out=acc[:, col:col + 1],
                    )
                    sq_insts[it].append(sq)
                    ex_insts[it].append(ex)

            # ---- event tails, at the END of every engine stream ----
            # gpsimd share: DVE product, PE channel reduce (single PSUM
            # tile, PSUM->SBUF relu copies between rounds), ACT sqrt
            _tt(nc, P[:], a_view, c_view, OP.mult)
            _tt(nc, P[:], P[:], t_view, OP.mult)
            q_ev = qpool.tile([128, N], F32, tag="q", name="q")
            ev_d2 = evpool.tile([8, EV_G], F32)
            pm = P[:].rearrange("p q j d -> p (q j d)")
            nmm = (EV_G + 511) // 512
            for r in range(nmm):
                c0 = (r % 4) * 512
                cw = min(512, EV_G - 512 * r)
                nc.tensor.matmul(
                    q_ev[0:8, c0:c0 + cw],
                    ones16[:],
                    pm[:, 512 * r:512 * r + cw],
                    start=True, stop=True,
                )
                nc.vector.tensor_scalar_max(
                    ev_d2[:, 512 * r:512 * r + cw],
                    q_ev[0:8, c0:c0 + cw],
                    0.0,
                )
            w_ev = evpool.tile([8, EV_G], BF16)
            ev_sq_g = nc.scalar.activation(
                w_ev[:], ev_d2[:], AF.Sqrt,
                accum_out=acc[0:8, 20:21],
            )

            # SWDGE share: distance algebra per chunk, then one sqrt
            for ch in range(EV_CHUNKS):
                emit_dma_event_math(ch, wpool)
            ev_sq_d = nc.scalar.activation(
                d_ev[:], d2all[:], AF.Sqrt, accum_out=acc[:, 21:22]
            )

            # ACT phase order: sqrt(i0) exp(i0) sqrt(i1) exp(i1) ev_g ev_d.
            # The event sqrts land last: their PE/DVE inputs are only
            # ready near the end of the main loop, and must not gate the
            # exp phases.
            order = (
                sq_insts[0] + ex_insts[0] + sq_insts[1] + ex_insts[1]
                + [ev_sq_g, ev_sq_d]
            )
            for a, b in zip(order[1:], order[:-1]):
                add_dep_helper(a.ins, b.ins, reason="act table phase order")

            nc.sync.dma_start(out_p, acc[:])

    nc.compile()
    _CACHE["nc"] = nc
    return nc


# trilinear channels: (A_k(u), B_k(t) power, C_k(v)); a = zx^2+zy^2,
# b = 2(zx vx + zy vy), c = vx^2+vy^2
# feature columns: [1, a, b, c, zx, zy, vx, vy]
_ACH = [1, 0, 4, 5, 2, 0, 4, 6, 5, 7, 3, 0, 6, 7]   # A feature index
_ASC = [1., 1., -2., -2., 1., 1., -2., -2., -2., -2., 1., 1., -2., -2.]
_BPOW = [0, 0, 0, 0, 1, 1, 1, 1, 1, 1, 2, 2, 2, 2]  # power of t
_CCH = [0, 1, 4, 5, 0, 2, 6, 4, 7, 5, 0, 3, 6, 7]   # C feature index


def _node_features(zv):
    zx, zy, vx, vy = zv[:, 0], zv[:, 1], zv[:, 2], zv[:, 3]
    a = zx * zx + zy * zy
    b = 2.0 * (zx * vx + zy * vy)
    c = vx * vx + vy * vy
    one = np.ones_like(a)
    return np.stack([one, a, b, c, zx, zy, vx, vy], axis=1)  # [N, 8]


def _marshal(inputs):
    import ml_dtypes

    z0 = np.asarray(inputs["z0"], dtype=np.float32)
    v0 = np.asarray(inputs["v0"], dtype=np.float32)
    uv = np.asarray(inputs["data_uv"], dtype=np.int32)
    tt = np.asarray(inputs["data_t"], dtype=np.float32)
    t0 = np.float32(np.asarray(inputs["t0"]).reshape(-1)[0])
    tn = np.float32(np.asarray(inputs["tn"]).reshape(-1)[0])

    zv = np.ascontiguousarray(np.concatenate([z0, v0], axis=1)).astype(np.float32)
    dt = np.float32((tn - t0) / np.float32(S))
    tmid = (t0 + (np.arange(S, dtype=np.float32) + np.float32(0.5)) * dt).astype(
        np.float32
    )
    tb = np.ascontiguousarray(np.broadcast_to(tmid, (128, S))).astype(np.float32)
    t2b = (tb * tb).astype(np.float32)

    zv_pad = np.zeros((N, GELEM), np.float32)
    zv_pad[:, 0:4] = zv

    feats = _node_features(zv.astype(np.float64)).astype(np.float32)  # [N, 8]
    gtab = np.zeros((128, N, 2), np.float32)
    for k in range(14):
        for g in range(NG):
            gtab[16 * g + k, :, 0] = _ASC[k] * feats[:, _ACH[k]]
            gtab[16 * g + k, :, 1] = feats[:, _CCH[k]]
    gtab = gtab.astype(ml_dtypes.bfloat16)

    E = uv.shape[0]
    assert E <= NCORES * EV_PER_CORE
    u_all = uv[:, 0].astype(np.int64)
    v_all = uv[:, 1].astype(np.int64)

    def split_core(u, v, t):
        """Assign each u-node's events wholly to the gpsimd or the SWDGE
        share; fill gpsimd groups (balanced) up to GP_TARGET events."""
        counts = np.bincount(u, minlength=N)
        order = np.argsort(-counts, kind="stable")
        g_ev = np.zeros(NG, np.int64)
        g_seg = np.zeros(NG, np.int64)
        node_g = np.full(N, -1, np.int64)   # -1 -> SWDGE share
        total = 0
        for n in order:
            c = int(counts[n])
            if c == 0 or total >= GP_TARGET:
                continue
            segs = -(-c // SLOT_G)
            g = int(np.argmin(g_ev))
            if g_ev[g] + segs * SLOT_G > EV_G or g_seg[g] + segs > SEG_G:
                continue
            node_g[n] = g
            g_ev[g] += segs * SLOT_G        # reserve whole segments
            g_seg[g] += segs
            total += c
        return node_g

    def pack_gp(u, v, t, node_g):
        """gpsimd share: wrapped gather index list + T tensor."""
        sel = node_g[u] >= 0
        us, vs, ts = u[sel], v[sel], t[sel]
        gs = node_g[us]
        order = np.argsort(us, kind="stable")
        us, vs, ts, gs = us[order], vs[order], ts[order], gs[order]

        seg_u = np.zeros((NG, SEG_G), np.int64)
        ev_vv = np.zeros((NG, SEG_G, SLOT_G), np.int64)
        ev_tt = np.zeros((NG, SEG_G, SLOT_G), np.float32)
        ev_mm = np.zeros((NG, SEG_G, SLOT_G), np.float32)
        seg_cnt = np.zeros(NG, np.int64)
        i = 0
        while i < len(us):
            j = i
            while j < len(us) and us[j] == us[i]:
                j += 1
            g = int(gs[i])
            for s0 in range(i, j, SLOT_G):
                q = seg_cnt[g]
                assert q < SEG_G, "gp segment overflow"
                seg_cnt[g] += 1
                e0 = min(s0 + SLOT_G, j)
                seg_u[g, q] = us[i]
                ev_vv[g, q, : e0 - s0] = vs[s0:e0]
                ev_tt[g, q, : e0 - s0] = ts[s0:e0]
                ev_mm[g, q, : e0 - s0] = 1.0
            i = j
        idx_flat = np.concatenate(
            [seg_u, ev_vv.reshape(NG, EV_G)], axis=1
        ).astype(np.int16)
        gidx = np.zeros((128, NIDX // 16), np.int16)
        for g in range(NG):
            gidx[16 * g:16 * (g + 1), :] = (
                idx_flat[g].reshape(NIDX // 16, 16).T
            )
        tmat = np.zeros((128, EV_G), np.float32)
        tflat = ev_tt.reshape(NG, EV_G)
        mflat = ev_mm.reshape(NG, EV_G)
        for k in range(14):
            p = _BPOW[k]
            for g in range(NG):
                tmat[16 * g + k, :] = (tflat[g] ** p) * mflat[g]
        return gidx, tmat.astype(ml_dtypes.bfloat16)

    def pack_dma(u, v, t, node_g):
        """SWDGE share: baseline segment layout (pads v=u, t=0)."""
        sel = node_g[u] < 0
        us, vs, ts = u[sel], v[sel], t[sel]
        order = np.argsort(us, kind="stable")
        us, vs, ts = us[order], vs[order], ts[order]
        starts = np.flatnonzero(np.r_[True, us[1:] != us[:-1]])
        ends = np.r_[starts[1:], len(us)]
        seg_nodes = np.zeros((128, SPD), np.int16)
        v_slots = np.zeros((128, SPD, SLOT_D), np.int16)
        t_slots = np.zeros((128, SPD, SLOT_D), np.float32)
        counts = np.zeros(128, np.int64)
        i = 0
        for s0, e0 in zip(starts, ends):
            n = us[s0]
            for j in range(s0, e0, SLOT_D):
                p = i % 128
                q = counts[p]
                counts[p] += 1
                assert q < SPD, "dma segment overflow; raise SPD"
                i += 1
                seg_nodes[p, q] = n
                va = vs[j:min(j + SLOT_D, e0)]
                ta = ts[j:min(j + SLOT_D, e0)]
                v_slots[p, q, :] = n
                v_slots[p, q, : len(va)] = va
                t_slots[p, q, : len(ta)] = ta
        return (
            seg_nodes,
            v_slots.reshape(128, C_EV),
            t_slots.reshape(128, C_EV),
        )

    def wrap16(x, nops, per_op):
        w = x.reshape(nops, per_op // 16, 16).transpose(2, 0, 1)
        return np.ascontiguousarray(np.tile(w, (8, 1, 1)))

    ones16 = np.zeros((128, 8), np.float32)
    for g in range(NG):
        ones16[16 * g:16 * (g + 1), g] = 1.0

    ident_np = np.eye(128, dtype=np.float32)
    in_maps = []
    _DBG_SPLIT.clear()
    for k in range(NCORES):
        sl = slice(k * EV_PER_CORE, (k + 1) * EV_PER_CORE)
        u, v, t = u_all[sl], v_all[sl], tt[sl]
        node_g = split_core(u, v, t)
        zv64 = zv.astype(np.float64)

        def _dsum(mask):
            uu, vv, tt_ = u[mask], v[mask], t[mask]
            dx = (zv64[uu, 0] - zv64[vv, 0]) + (zv64[uu, 2] - zv64[vv, 2]) * tt_
            dy = (zv64[uu, 1] - zv64[vv, 1]) + (zv64[uu, 3] - zv64[vv, 3]) * tt_
            return float(np.sqrt(dx * dx + dy * dy).sum())

        _DBG_SPLIT.append(
            (_dsum(node_g[u] >= 0), _dsum(node_g[u] < 0), int((node_g[u] >= 0).sum()))
        )
        gidx, tmat = pack_gp(u, v, t, node_g)
        seg_nodes, v_slots, t_slots = pack_dma(u, v, t, node_g)
        seg_list = seg_nodes.T.reshape(-1)
        v_list = (
            v_slots.reshape(128, EV_CHUNKS, EV_CC)
            .transpose(1, 2, 0)
            .reshape(-1)
        )
        zvi = zv[k * 256:(k + 1) * 256]
        fblob = np.concatenate(
            [
                zv.reshape(16, 128, 4).transpose(1, 0, 2).reshape(128, 64),
                zvi.reshape(2, 128, 4).transpose(1, 0, 2).reshape(128, 8),
                tb,
                t2b,
                ident_np,
                ones16,
                t_slots,
            ],
            axis=1,
        ).astype(np.float32)
        bblob = np.concatenate(
            [gtab.reshape(128, 2 * N), tmat], axis=1
        )
        iblob = np.concatenate(
            [
                wrap16(seg_list, SEG_OPS, SEG_PER_OP).reshape(128, -1),
                wrap16(v_list, EV_CHUNKS, EV_PER_CHUNK).reshape(128, -1),
                gidx,
            ],
            axis=1,
        ).astype(np.int16)
        in_maps.append(
            {
                "zv_pad": zv_pad,
                "fblob": np.ascontiguousarray(fblob),
                "bblob": np.ascontiguousarray(bblob),
                "iblob": np.ascontiguousarray(iblob),
            }
        )
    return in_maps, (float(t0), float(tn), E)


def _np_event_total(inputs, core):
    """float64 reference event-distance sum for one core's slice."""
    z0 = np.asarray(inputs["z0"], np.float64)
    v0 = np.asarray(inputs["v0"], np.float64)
    uv = np.asarray(inputs["data_uv"], np.int64)
    tt = np.asarray(inputs["data_t"], np.float64)
    sl = slice(core * EV_PER_CORE, (core + 1) * EV_PER_CORE)
    u, v, t = uv[sl, 0], uv[sl, 1], tt[sl]
    dx = (z0[u, 0] - z0[v, 0]) + (v0[u, 0] - v0[v, 0]) * t
    dy = (z0[u, 1] - z0[v, 1]) + (v0[u, 1] - v0[v, 1]) * t
    return np.sqrt(dx * dx + dy * dy).sum()


def _combine(core_outs, beta, t0, tn, E):
    """core_outs: list of [128, 24] float32 partial-sum tensors."""
    exp_sum = 0.0
    ev_sum = 0.0
    for o in core_outs:
        o = np.asarray(o, dtype=np.float64)
        exp_sum += o[:, 0 : ITILES * S].sum()
        ev_sum += o[:, 20].sum() + o[:, 21].sum()
    b = float(beta)
    dt = (tn - t0) / S
    event_intensity = E * b - ev_sum
    non_event = np.exp(b) * (exp_sum - S * N) / 2.0 * dt
    return np.float32(event_intensity - 1.0 * non_event)


def kernel(**inputs) -> np.ndarray:
    from concourse.bass_utils import run_bass_kernel_spmd

    nc = _build()
    in_maps, (t0, tn, E) = _marshal(inputs)
    res = run_bass_kernel_spmd(nc, in_maps, core_ids=list(range(NCORES)))
    beta = float(np.asarray(inputs["beta"]).reshape(-1)[0])
    out = _combine([r["out_p"] for r in res.results], beta, t0, tn, E)
    return np.asarray(out, dtype=np.float32)


# revision 38
# speedup vs baseline: 1.2646x; 1.1607x over previous
"""Trainium2 Bass kernel for BasicEuclideanDistModel (gnn_message_passing).

Math:
  result = sum_e (beta - ||dz_e + dv_e t_e||)
           - dt * sum_{i<j, s} exp(beta - ||z_i(t_s) - z_j(t_s)||)

Device strategy (8 cores, data parallel):
  * Non-event term: full NxN pairwise distances (halved on host).
    d^2(i,j,s) = F_i(s) . G_j  (K=8 inner product, G time-independent).
    One [8,128]x[8,2048] matmul (fp32r) per (i-tile, s) computes the
    d^2 supertile; DVE relu clamps rounding negatives, ACT computes
    sqrt then exp(-d) with fused row sums.  Each core owns 2 of the
    16 i-tiles, all j, all 10 samples.
  * Event term, split across two independent engines working in
    parallel (events of one u-node always stay together):
    - gpsimd share: d^2(u,v,t) = sum_k A_k(u) B_k(t) C_k(v), a
      14-channel trilinear decomposition with B_k in {1,t,t^2}.
      Events form 8 groups (one per Q7 tile); partition 16g+k holds
      channel k.  ONE ap_gather (SBUF gather, ~27.5ns/idx/core,
      shared index list per group) fetches A_k per segment and C_k
      per event from a [128, N, 2] bf16 channel table; DVE forms
      P = A*C*T (host T = B_k(t)*mask), PE reduces channels with a
      block-ones stationary, one ACT sqrt row-sum -> acc col 20.
    - SWDGE share: baseline scheme -- events grouped by u into
      segments laid out [128, SPD, SLOTD]; dma_gather fetches 256B
      rows (u per segment, v per slot, ~3.8ns/desc aggregate); DVE
      distance algebra, ACT sqrt row-sum -> acc col 21.  Pad slots
      use v=u, t=0 (exactly 0 contribution).
  * beta enters only as a scalar factor / offset -> folded in on host.
  Host combines 8 cores' [128, 24] partial-sum tensors (pure unshard/
  reduction of partials).
"""

import os
import numpy as np


def _import_concourse():
    try:
        import concourse  # noqa: F401
    except ImportError:
        import sys

        for p in ("/opt/trn_rl_repo", "/root/.axon_site/_ro/trn_rl_repo"):
            if os.path.isdir(p) and p not in sys.path:
                sys.path.insert(0, p)


_import_concourse()

from contextlib import ExitStack  # noqa: E402

import concourse.bacc as bacc  # noqa: E402
import concourse.mybir as mybir  # noqa: E402
import concourse.tile as tile  # noqa: E402
from concourse.tile_rust import add_dep_helper  # noqa: E402

N = 2048          # nodes
S = 10            # Riemann samples
NCORES = 8
ITILES = 2        # 128-row i-tiles per core
EV_PER_CORE = 200000 // NCORES       # real events per core

# ---- gpsimd (ap_gather) event share: the highest-count u-nodes ----
NG = 8            # groups (one per Q7 tile of 16 partitions)
SLOT_G = 6        # event slots per segment
SEG_G = 208       # segments per group
EV_G = SEG_G * SLOT_G                # 1248 event slots per group
NIDX = SEG_G + EV_G                  # 1456 gather indices per group
GP_TARGET = 0                        # target events on the gpsimd side

# ---- SWDGE (dma_gather) event share ----
SLOT_D = 6        # event slots per segment
SPD = 40          # segments per partition
C_EV = SPD * SLOT_D                  # 168 event columns per partition
NSEG = 128 * SPD                     # 3584 segments per core
SEG_OPS = 2       # seg-gather split into this many dma_gather ops
SEG_PER_OP = NSEG // SEG_OPS
EV_CHUNKS = 4     # v-side gather ops (one per SWDGE queue)
EV_CC = C_EV // EV_CHUNKS            # 42 event columns per chunk
EV_PER_CHUNK = 128 * EV_CC
GELEM = 64        # gather element size in f32 (256B rows)

F32 = mybir.dt.float32
F32R = mybir.dt.float32r
BF16 = mybir.dt.bfloat16
I16 = mybir.dt.int16
AF = mybir.ActivationFunctionType
OP = mybir.AluOpType

_CACHE: dict = {}
_DBG_SPLIT: list = []


def _tt(nc, out, in0, in1, op):
    return nc.vector.tensor_tensor(out, in0, in1, op=op)


def _build():
    if "nc" in _CACHE:
        return _CACHE["nc"]

    nc = bacc.Bacc(
        "TRN2", target_bir_lowering=False, debug=False, enable_asserts=False,
        num_swdge_queues=4,
    )

    # inputs coalesced into 3 blobs: per-DMA fixed cost (~2.4us) made 19
    # separate loads a ~46us critical-path prefix
    FB = 228 + C_EV   # f32 blob: zv(64) zvi(8) tb(10) t2b(10) ident(128)
    #                   ones16(8) ev_t(C_EV)
    BB = 2 * N + EV_G                # bf16 blob: gtab (N x 2) | tmat
    IB = (SEG_OPS * SEG_PER_OP + EV_CHUNKS * EV_PER_CHUNK + NIDX) // 16
    zv_pad = nc.dram_tensor("zv_pad", [N, GELEM], F32, kind="ExternalInput").ap()
    fblob_d = nc.dram_tensor("fblob", [128, FB], F32, kind="ExternalInput").ap()
    bblob_d = nc.dram_tensor("bblob", [128, BB], BF16, kind="ExternalInput").ap()
    iblob_d = nc.dram_tensor("iblob", [128, IB], I16, kind="ExternalInput").ap()
    out_p = nc.dram_tensor("out_p", [128, 24], F32, kind="ExternalOutput").ap()

    with tile.TileContext(nc) as tc, ExitStack() as ctx:
        cpool = ctx.enter_context(tc.tile_pool(name="const", bufs=1))
        evpool = ctx.enter_context(tc.tile_pool(name="ev", bufs=1))

        # ---------------- input loads (3 coalesced blobs) ----------------
        ib_sb = evpool.tile([128, IB], I16)
        nc.sync.dma_start(ib_sb[:], iblob_d)
        fb_sb = cpool.tile([128, FB], F32)
        nc.sync.dma_start(fb_sb[:], fblob_d)
        bb_sb = evpool.tile([128, BB], BF16)
        nc.sync.dma_start(bb_sb[:], bblob_d)

        UW = SEG_PER_OP // 16
        VW = EV_PER_CHUNK // 16
        u_sb = ib_sb[:, 0:SEG_OPS * UW].rearrange("p (a b) -> p a b", a=SEG_OPS)
        v_sb = ib_sb[:, SEG_OPS * UW:SEG_OPS * UW + EV_CHUNKS * VW].rearrange(
            "p (a b) -> p a b", a=EV_CHUNKS
        )
        gidx = ib_sb[:, SEG_OPS * UW + EV_CHUNKS * VW:IB]
        zv_sb = fb_sb[:, 0:64].rearrange("p (c d) -> p c d", d=4)
        zvi_sb = fb_sb[:, 64:72].rearrange("p (c d) -> p c d", d=4)
        tb = fb_sb[:, 72:82]
        t2b = fb_sb[:, 82:92]
        ident = fb_sb[:, 92:220]
        ones16_f = fb_sb[:, 220:228]
        t_sb = fb_sb[:, 228:228 + C_EV]
        gtab = bb_sb[:, 0:2 * N].rearrange("p (n d) -> p n d", d=2)
        tmat = bb_sb[:, 2 * N:BB]

        acc = cpool.tile([128, 24], F32)
        nc.vector.memset(acc[:], 0.0)

        # ---------------- event gathers ----------------
        # gpsimd stream order matters: the SWDGE descriptor GENERATION
        # runs first (its DMA drain proceeds on the DMA engines while the
        # gpsimd engine moves on), then one library reload, then the
        # ap_gather for the gpsimd share.
        # SWDGE share: u-side one 256B row per SEGMENT; v-side one row
        # per event slot (pads gather v=u, t=0 -> exactly 0)
        d2all = evpool.tile([128, C_EV, 1], F32)
        seg = evpool.tile([128, SPD, GELEM], F32)
        for so in range(SEG_OPS):
            nc.gpsimd.dma_gather(
                seg[:, so * (SPD // SEG_OPS):(so + 1) * (SPD // SEG_OPS), :],
                zv_pad, u_sb[:, so, :], SEG_PER_OP, SEG_PER_OP, GELEM,
                single_packet=False, queue_num=so % 4,
            )
        dvg = ctx.enter_context(tc.tile_pool(name="dvg", bufs=4))
        b_tiles = []
        last_gather = None
        for ch in range(EV_CHUNKS):
            B = dvg.tile([128, EV_CC, GELEM], F32, tag="B", name="B")
            last_gather = nc.gpsimd.dma_gather(
                B[:], zv_pad, v_sb[:, ch, :], EV_PER_CHUNK, EV_PER_CHUNK, GELEM,
                single_packet=False, queue_num=ch % 4,
            )
            b_tiles.append(B)

        # gpsimd share: one ap_gather; table row n = [A_k(n), C_k(n)]
        # (d=2 bf16 = 4B per index); segment slots use component 0,
        # event slots component 1
        if GP_TARGET:
            gout = evpool.tile([128, NIDX, 2], BF16)
            nc.gpsimd.ap_gather(
                gout[:], gtab, gidx,
                channels=128, num_elems=N, d=2, num_idxs=NIDX,
            )
            ones16 = evpool.tile([128, 8], BF16)
            nc.vector.tensor_copy(ones16[:], ones16_f)
        _ = last_gather

        def emit_dma_event_math(ch, scratch_pool):
            B = b_tiles[ch]
            sc = EV_CC // SLOT_D                     # segments per chunk
            q0 = ch * sc
            shape4 = [128, sc, SLOT_D, 1]
            tse = (
                t_sb[:, ch * EV_CC:(ch + 1) * EV_CC]
                .rearrange("p (q j) -> p q j", j=SLOT_D)
                .unsqueeze(3)
            )

            def sv(d):  # seg channel d view broadcast over the slots
                return (
                    seg[:, q0:q0 + sc, d:d + 1]
                    .unsqueeze(2)
                    .to_broadcast(shape4)
                )

            def bv(d):  # B channel d view
                return B[:, :, d:d + 1].rearrange(
                    "p (q j) d -> p q j d", j=SLOT_D
                )

            dzx = scratch_pool.tile(shape4, F32, tag="w", name="dzx")
            dvx = scratch_pool.tile(shape4, F32, tag="w", name="dvx")
            dzy = scratch_pool.tile(shape4, F32, tag="w", name="dzy")
            dvy = scratch_pool.tile(shape4, F32, tag="w", name="dvy")
            _tt(nc, dzx[:], sv(0), bv(0), OP.subtract)
            _tt(nc, dvx[:], sv(2), bv(2), OP.subtract)
            _tt(nc, dvx[:], dvx[:], tse, OP.mult)
            _tt(nc, dzx[:], dzx[:], dvx[:], OP.add)          # dx
            _tt(nc, dzy[:], sv(1), bv(1), OP.subtract)
            _tt(nc, dvy[:], sv(3), bv(3), OP.subtract)
            _tt(nc, dvy[:], dvy[:], tse, OP.mult)
            _tt(nc, dzy[:], dzy[:], dvy[:], OP.add)          # dy
            _tt(nc, dzx[:], dzx[:], dzx[:], OP.mult)
            _tt(nc, dzy[:], dzy[:], dzy[:], OP.mult)
            d2v = d2all[:, ch * EV_CC:(ch + 1) * EV_CC, :].rearrange(
                "p (q j) d -> p q j d", j=SLOT_D
            )
            _tt(nc, d2v, dzx[:], dzy[:], OP.add)             # d^2

        # ---------------- j features  F[p, chunk, 0:8] ----------------
        # [1, a, b, c, zx, vx, zy, vy]; padded to 32 for the PE transpose
        F = cpool.tile([128, 16, 32], F32)
        zx = zv_sb[:, :, 0:1]
        zy = zv_sb[:, :, 1:2]
        vx = zv_sb[:, :, 2:3]
        vy = zv_sb[:, :, 3:4]
        s1 = cpool.tile([128, 16, 1], F32)
        nc.vector.memset(F[:, :, 0:1], 1.0)
        _tt(nc, F[:, :, 1:2], zx, zx, OP.mult)           # a = zx^2 + zy^2
        _tt(nc, s1[:], zy, zy, OP.mult)
        _tt(nc, F[:, :, 1:2], F[:, :, 1:2], s1[:], OP.add)
        s2 = cpool.tile([128, 16, 1], F32)
        _tt(nc, F[:, :, 2:3], zx, vx, OP.mult)           # b = 2(zx vx + zy vy)
        _tt(nc, s2[:], zy, vy, OP.mult)
        _tt(nc, F[:, :, 2:3], F[:, :, 2:3], s2[:], OP.add)
        nc.vector.tensor_scalar_mul(F[:, :, 2:3], F[:, :, 2:3], 2.0)
        s3 = cpool.tile([128, 16, 1], F32)
        _tt(nc, F[:, :, 3:4], vx, vx, OP.mult)           # c = vx^2 + vy^2
        _tt(nc, s3[:], vy, vy, OP.mult)
        _tt(nc, F[:, :, 3:4], F[:, :, 3:4], s3[:], OP.add)
        nc.vector.tensor_copy(F[:, :, 4:5], zx)
        nc.vector.tensor_copy(F[:, :, 5:6], vx)
        nc.vector.tensor_copy(F[:, :, 6:7], zy)
        nc.vector.tensor_copy(F[:, :, 7:8], vy)

        # ---------------- i features  L[p, it, s, 0:8] ----------------
        # [r, 1, t, t^2, -2x, -2tx, -2y, -2ty]
        L = cpool.tile([128, ITILES, S, 32], F32)
        izx = zvi_sb[:, :, 0:1]
        izy = zvi_sb[:, :, 1:2]
        ivx = zvi_sb[:, :, 2:3]
        ivy = zvi_sb[:, :, 3:4]
        ia = cpool.tile([128, ITILES, 1], F32)
        ib = cpool.tile([128, ITILES, 1], F32)
        ic = cpool.tile([128, ITILES, 1], F32)
        s4 = cpool.tile([128, ITILES, 1], F32)
        _tt(nc, ia[:], izx, izx, OP.mult)
        _tt(nc, s4[:], izy, izy, OP.mult)
        _tt(nc, ia[:], ia[:], s4[:], OP.add)
        s5 = cpool.tile([128, ITILES, 1], F32)
        _tt(nc, ib[:], izx, ivx, OP.mult)
        _tt(nc, s5[:], izy, ivy, OP.mult)
        _tt(nc, ib[:], ib[:], s5[:], OP.add)
        nc.vector.tensor_scalar_mul(ib[:], ib[:], 2.0)
        s6 = cpool.tile([128, ITILES, 1], F32)
        _tt(nc, ic[:], ivx, ivx, OP.mult)
        _tt(nc, s6[:], ivy, ivy, OP.mult)
        _tt(nc, ic[:], ic[:], s6[:], OP.add)

        def b_i(v):  # [128, ITILES, 1] -> [128, ITILES, S, 1]
            return v.unsqueeze(2).to_broadcast([128, ITILES, S, 1])

        tv = tb.unsqueeze(1).unsqueeze(3).to_broadcast([128, ITILES, S, 1])
        t2v = t2b.unsqueeze(1).unsqueeze(3).to_broadcast([128, ITILES, S, 1])

        nc.vector.memset(L[:, :, :, 1:2], 1.0)
        nc.vector.tensor_copy(L[:, :, :, 2:3], tv)
        nc.vector.tensor_copy(L[:, :, :, 3:4], t2v)
        Lx = cpool.tile([128, ITILES, S, 1], F32)
        _tt(nc, Lx[:], b_i(ivx), tv, OP.mult)            # x_i(s) = zx + vx t
        _tt(nc, Lx[:], Lx[:], b_i(izx), OP.add)
        nc.vector.tensor_scalar_mul(L[:, :, :, 4:5], Lx[:], -2.0)
        _tt(nc, L[:, :, :, 5:6], L[:, :, :, 4:5], tv, OP.mult)
        Ly = cpool.tile([128, ITILES, S, 1], F32)
        _tt(nc, Ly[:], b_i(ivy), tv, OP.mult)
        _tt(nc, Ly[:], Ly[:], b_i(izy), OP.add)
        nc.vector.tensor_scalar_mul(L[:, :, :, 6:7], Ly[:], -2.0)
        _tt(nc, L[:, :, :, 7:8], L[:, :, :, 6:7], tv, OP.mult)
        Lr = cpool.tile([128, ITILES, S, 1], F32)
        _tt(nc, L[:, :, :, 0:1], b_i(ib), tv, OP.mult)   # r = a + b t + c t^2
        _tt(nc, L[:, :, :, 0:1], L[:, :, :, 0:1], b_i(ia), OP.add)
        _tt(nc, Lr[:], b_i(ic), t2v, OP.mult)
        _tt(nc, L[:, :, :, 0:1], L[:, :, :, 0:1], Lr[:], OP.add)

        # ---------------- transposes (PE) ----------------
        T2 = cpool.tile([8, N], F32R)                    # G_j rows
        L2 = cpool.tile([8, ITILES * S, 128], F32R)      # F_i(s) rows
        with tc.tile_pool(name="tp", bufs=4, space="PSUM") as tpp:
            for c in range(16):
                pt = tpp.tile([32, 128], F32, tag="pt", name="pt")
                nc.tensor.transpose(pt[:], F[:, c, :], ident)
                nc.vector.tensor_copy(T2[:, c * 128:(c + 1) * 128], pt[0:8, :])
            for it in range(ITILES):
                for s in range(S):
                    pt = tpp.tile([32, 128], F32, tag="pt", name="pt")
                    nc.tensor.transpose(pt[:], L[:, it, s, :], ident)
                    nc.vector.tensor_copy(L2[:, it * S + s, :], pt[0:8, :])

        # gpsimd-share P = A * C * T views
        if GP_TARGET:
            P = evpool.tile([128, SEG_G, SLOT_G, 1], BF16)
            shape4g = [128, SEG_G, SLOT_G, 1]
            a_view = gout[:, 0:SEG_G, 0:1].unsqueeze(2).to_broadcast(shape4g)
            c_view = gout[:, SEG_G:NIDX, 1:2].rearrange(
                "p (q j) d -> p q j d", j=SLOT_G
            )
            t_view = tmat.rearrange("p (q j) -> p q j", j=SLOT_G).unsqueeze(3)
        d_ev = evpool.tile([128, C_EV, 1], F32)

        # ---------------- main pairwise loop ----------------
        sq_insts = [[] for _ in range(ITILES)]
        ex_insts = [[] for _ in range(ITILES)]
        with tc.tile_pool(name="qp", bufs=2, space="PSUM") as qpool, \
                tc.tile_pool(name="wp", bufs=12) as wpool:
            for it in range(ITILES):
                for s in range(S):
                    q = qpool.tile([128, N], F32, tag="q", name="q")
                    for kk in range(4):
                        nc.tensor.matmul(
                            q[:, kk * 512:(kk + 1) * 512],
                            L2[:, it * S + s, :],
                            T2[:, kk * 512:(kk + 1) * 512],
                            start=True, stop=True,
                        )
                    w = wpool.tile([128, N], BF16, tag="w", name="w")
                    nc.vector.tensor_scalar_max(w[:], q[:], 0.0)
                    col = it * S + s
                    sq = nc.scalar.activation(w[:], w[:], AF.Sqrt)
                    ex = nc.scalar.activation(
                        w[:], w[:], AF.Exp, scale=-1.0,
                        accum_out=acc[:, col:col + 1],
                    )
                    sq_insts[it].append(sq)
                    ex_insts[it].append(ex)

            # ---- event tails, at the END of every engine stream ----
            # gpsimd share: DVE product, PE channel reduce (single PSUM
            # tile, PSUM->SBUF relu copies between rounds), ACT sqrt
            ev_tail = []
            if GP_TARGET:
                _tt(nc, P[:], a_view, c_view, OP.mult)
                _tt(nc, P[:], P[:], t_view, OP.mult)
                q_ev = qpool.tile([128, N], F32, tag="q", name="q")
                ev_d2 = evpool.tile([8, EV_G], F32)
                pm = P[:].rearrange("p q j d -> p (q j d)")
                nmm = (EV_G + 511) // 512
                for r in range(nmm):
                    c0 = (r % 4) * 512
                    cw = min(512, EV_G - 512 * r)
                    nc.tensor.matmul(
                        q_ev[0:8, c0:c0 + cw],
                        ones16[:],
                        pm[:, 512 * r:512 * r + cw],
                        start=True, stop=True,
                    )
                    nc.vector.tensor_scalar_max(
                        ev_d2[:, 512 * r:512 * r + cw],
                        q_ev[0:8, c0:c0 + cw],
                        0.0,
                    )
                w_ev = evpool.tile([8, EV_G], BF16)
                ev_tail.append(nc.scalar.activation(
                    w_ev[:], ev_d2[:], AF.Sqrt,
                    accum_out=acc[0:8, 20:21],
                ))

            # SWDGE share: distance algebra per chunk, then one sqrt
            for ch in range(EV_CHUNKS):
                emit_dma_event_math(ch, wpool)
            ev_tail.append(nc.scalar.activation(
                d_ev[:], d2all[:], AF.Sqrt, accum_out=acc[:, 21:22]
            ))

            # ACT phase order: sqrt(i0) exp(i0) sqrt(i1) exp(i1) ev_g ev_d.
            # The event sqrts land last: their PE/DVE inputs are only
            # ready near the end of the main loop, and must not gate the
            # exp phases.
            order = (
                sq_insts[0] + ex_insts[0] + sq_insts[1] + ex_insts[1]
                + ev_tail
            )
            for a, b in zip(order[1:], order[:-1]):
                add_dep_helper(a.ins, b.ins, reason="act table phase order")

            nc.sync.dma_start(out_p, acc[:])

    nc.compile()
    _CACHE["nc"] = nc
    return nc


# trilinear channels: (A_k(u), B_k(t) power, C_k(v)); a = zx^2+zy^2,
# b = 2(zx vx + zy vy), c = vx^2+vy^2
# feature columns: [1, a, b, c, zx, zy, vx, vy]
_ACH = [1, 0, 4, 5, 2, 0, 4, 6, 5, 7, 3, 0, 6, 7]   # A feature index
_ASC = [1., 1., -2., -2., 1., 1., -2., -2., -2., -2., 1., 1., -2., -2.]
_BPOW = [0, 0, 0, 0, 1, 1, 1, 1, 1, 1, 2, 2, 2, 2]  # power of t
_CCH = [0, 1, 4, 5, 0, 2, 6, 4, 7, 5, 0, 3, 6, 7]   # C feature index


def _node_features(zv):
    zx, zy, vx, vy = zv[:, 0], zv[:, 1], zv[:, 2], zv[:, 3]
    a = zx * zx + zy * zy
    b = 2.0 * (zx * vx + zy * vy)
    c = vx * vx + vy * vy
    one = np.ones_like(a)
    return np.stack([one, a, b, c, zx, zy, vx, vy], axis=1)  # [N, 8]


def _marshal(inputs):
    import ml_dtypes

    z0 = np.asarray(inputs["z0"], dtype=np.float32)
    v0 = np.asarray(inputs["v0"], dtype=np.float32)
    uv = np.asarray(inputs["data_uv"], dtype=np.int32)
    tt = np.asarray(inputs["data_t"], dtype=np.float32)
    t0 = np.float32(np.asarray(inputs["t0"]).reshape(-1)[0])
    tn = np.float32(np.asarray(inputs["tn"]).reshape(-1)[0])

    zv = np.ascontiguousarray(np.concatenate([z0, v0], axis=1)).astype(np.float32)
    dt = np.float32((tn - t0) / np.float32(S))
    tmid = (t0 + (np.arange(S, dtype=np.float32) + np.float32(0.5)) * dt).astype(
        np.float32
    )
    tb = np.ascontiguousarray(np.broadcast_to(tmid, (128, S))).astype(np.float32)
    t2b = (tb * tb).astype(np.float32)

    zv_pad = np.zeros((N, GELEM), np.float32)
    zv_pad[:, 0:4] = zv

    feats = _node_features(zv.astype(np.float64)).astype(np.float32)  # [N, 8]
    gtab = np.zeros((128, N, 2), np.float32)
    for k in range(14):
        for g in range(NG):
            gtab[16 * g + k, :, 0] = _ASC[k] * feats[:, _ACH[k]]
            gtab[16 * g + k, :, 1] = feats[:, _CCH[k]]
    gtab = gtab.astype(ml_dtypes.bfloat16)

    E = uv.shape[0]
    assert E <= NCORES * EV_PER_CORE
    u_all = uv[:, 0].astype(np.int64)
    v_all = uv[:, 1].astype(np.int64)

    def split_core(u, v, t):
        """Assign each u-node's events wholly to the gpsimd or the SWDGE
        share; fill gpsimd groups (balanced) up to GP_TARGET events."""
        counts = np.bincount(u, minlength=N)
        order = np.argsort(-counts, kind="stable")
        g_ev = np.zeros(NG, np.int64)
        g_seg = np.zeros(NG, np.int64)
        node_g = np.full(N, -1, np.int64)   # -1 -> SWDGE share
        total = 0
        for n in order:
            c = int(counts[n])
            if c == 0 or total >= GP_TARGET:
                continue
            segs = -(-c // SLOT_G)
            g = int(np.argmin(g_ev))
            if g_ev[g] + segs * SLOT_G > EV_G or g_seg[g] + segs > SEG_G:
                continue
            node_g[n] = g
            g_ev[g] += segs * SLOT_G        # reserve whole segments
            g_seg[g] += segs
            total += c
        return node_g

    def pack_gp(u, v, t, node_g):
        """gpsimd share: wrapped gather index list + T tensor."""
        sel = node_g[u] >= 0
        us, vs, ts = u[sel], v[sel], t[sel]
        gs = node_g[us]
        order = np.argsort(us, kind="stable")
        us, vs, ts, gs = us[order], vs[order], ts[order], gs[order]

        seg_u = np.zeros((NG, SEG_G), np.int64)
        ev_vv = np.zeros((NG, SEG_G, SLOT_G), np.int64)
        ev_tt = np.zeros((NG, SEG_G, SLOT_G), np.float32)
        ev_mm = np.zeros((NG, SEG_G, SLOT_G), np.float32)
        seg_cnt = np.zeros(NG, np.int64)
        i = 0
        while i < len(us):
            j = i
            while j < len(us) and us[j] == us[i]:
                j += 1
            g = int(gs[i])
            for s0 in range(i, j, SLOT_G):
                q = seg_cnt[g]
                assert q < SEG_G, "gp segment overflow"
                seg_cnt[g] += 1
                e0 = min(s0 + SLOT_G, j)
                seg_u[g, q] = us[i]
                ev_vv[g, q, : e0 - s0] = vs[s0:e0]
                ev_tt[g, q, : e0 - s0] = ts[s0:e0]
                ev_mm[g, q, : e0 - s0] = 1.0
            i = j
        idx_flat = np.concatenate(
            [seg_u, ev_vv.reshape(NG, EV_G)], axis=1
        ).astype(np.int16)
        gidx = np.zeros((128, NIDX // 16), np.int16)
        for g in range(NG):
            gidx[16 * g:16 * (g + 1), :] = (
                idx_flat[g].reshape(NIDX // 16, 16).T
            )
        tmat = np.zeros((128, EV_G), np.float32)
        tflat = ev_tt.reshape(NG, EV_G)
        mflat = ev_mm.reshape(NG, EV_G)
        for k in range(14):
            p = _BPOW[k]
            for g in range(NG):
                tmat[16 * g + k, :] = (tflat[g] ** p) * mflat[g]
        return gidx, tmat.astype(ml_dtypes.bfloat16)

    def pack_dma(u, v, t, node_g):
        """SWDGE share: baseline segment layout (pads v=u, t=0)."""
        sel = node_g[u] < 0
        us, vs, ts = u[sel], v[sel], t[sel]
        order = np.argsort(us, kind="stable")
        us, vs, ts = us[order], vs[order], ts[order]
        starts = np.flatnonzero(np.r_[True, us[1:] != us[:-1]])
        ends = np.r_[starts[1:], len(us)]
        seg_nodes = np.zeros((128, SPD), np.int16)
        v_slots = np.zeros((128, SPD, SLOT_D), np.int16)
        t_slots = np.zeros((128, SPD, SLOT_D), np.float32)
        counts = np.zeros(128, np.int64)
        i = 0
        for s0, e0 in zip(starts, ends):
            n = us[s0]
            for j in range(s0, e0, SLOT_D):
                p = i % 128
                q = counts[p]
                counts[p] += 1
                assert q < SPD, "dma segment overflow; raise SPD"
                i += 1
                seg_nodes[p, q] = n
                va = vs[j:min(j + SLOT_D, e0)]
                ta = ts[j:min(j + SLOT_D, e0)]
                v_slots[p, q, :] = n
                v_slots[p, q, : len(va)] = va
                t_slots[p, q, : len(ta)] = ta
        return (
            seg_nodes,
            v_slots.reshape(128, C_EV),
            t_slots.reshape(128, C_EV),
        )

    def wrap16(x, nops, per_op):
        w = x.reshape(nops, per_op // 16, 16).transpose(2, 0, 1)
        return np.ascontiguousarray(np.tile(w, (8, 1, 1)))

    ones16 = np.zeros((128, 8), np.float32)
    for g in range(NG):
        ones16[16 * g:16 * (g + 1), g] = 1.0

    ident_np = np.eye(128, dtype=np.float32)
    in_maps = []
    _DBG_SPLIT.clear()
    for k in range(NCORES):
        sl = slice(k * EV_PER_CORE, (k + 1) * EV_PER_CORE)
        u, v, t = u_all[sl], v_all[sl], tt[sl]
        node_g = split_core(u, v, t)
        zv64 = zv.astype(np.float64)

        def _dsum(mask):
            uu, vv, tt_ = u[mask], v[mask], t[mask]
            dx = (zv64[uu, 0] - zv64[vv, 0]) + (zv64[uu, 2] - zv64[vv, 2]) * tt_
            dy = (zv64[uu, 1] - zv64[vv, 1]) + (zv64[uu, 3] - zv64[vv, 3]) * tt_
            return float(np.sqrt(dx * dx + dy * dy).sum())

        _DBG_SPLIT.append(
            (_dsum(node_g[u] >= 0), _dsum(node_g[u] < 0), int((node_g[u] >= 0).sum()))
        )
        gidx, tmat = pack_gp(u, v, t, node_g)
        seg_nodes, v_slots, t_slots = pack_dma(u, v, t, node_g)
        seg_list = seg_nodes.T.reshape(-1)
        v_list = (
            v_slots.reshape(128, EV_CHUNKS, EV_CC)
            .transpose(1, 2, 0)
            .reshape(-1)
        )
        zvi = zv[k * 256:(k + 1) * 256]
        fblob = np.concatenate(
            [
                zv.reshape(16, 128, 4).transpose(1, 0, 2).reshape(128, 64),
                zvi.reshape(2, 128, 4).transpose(1, 0, 2).reshape(128, 8),
                tb,
                t2b,
                ident_np,
                ones16,
                t_slots,
            ],
            axis=1,
        ).astype(np.float32)
        bblob = np.concatenate(
            [gtab.reshape(128, 2 * N), tmat], axis=1
        )
        iblob = np.concatenate(
            [
                wrap16(seg_list, SEG_OPS, SEG_PER_OP).reshape(128, -1),
                wrap16(v_list, EV_CHUNKS, EV_PER_CHUNK).reshape(128, -1),
                gidx,
            ],
            axis=1,
        ).astype(np.int16)
        in_maps.append(
            {
                "zv_pad": zv_pad,
                "fblob": np.ascontiguousarray(fblob),
                "bblob": np.ascontiguousarray(bblob),
                "iblob": np.ascontiguousarray(iblob),
            }
        )
    return in_maps, (float(t0), float(tn), E)


def _np_event_total(inputs, core):
    """float64 reference event-distance sum for one core's slice."""
    z0 = np.asarray(inputs["z0"], np.float64)
    v0 = np.asarray(inputs["v0"], np.float64)
    uv = np.asarray(inputs["data_uv"], np.int64)
    tt = np.asarray(inputs["data_t"], np.float64)
    sl = slice(core * EV_PER_CORE, (core + 1) * EV_PER_CORE)
    u, v, t = uv[sl, 0], uv[sl, 1], tt[sl]
    dx = (z0[u, 0] - z0[v, 0]) + (v0[u, 0] - v0[v, 0]) * t
    dy = (z0[u, 1] - z0[v, 1]) + (v0[u, 1] - v0[v, 1]) * t
    return np.sqrt(dx * dx + dy * dy).sum()


def _combine(core_outs, beta, t0, tn, E):
    """core_outs: list of [128, 24] float32 partial-sum tensors."""
    exp_sum = 0.0
    ev_sum = 0.0
    for o in core_outs:
        o = np.asarray(o, dtype=np.float64)
        exp_sum += o[:, 0 : ITILES * S].sum()
        ev_sum += o[:, 20].sum() + o[:, 21].sum()
    b = float(beta)
    dt = (tn - t0) / S
    event_intensity = E * b - ev_sum
    non_event = np.exp(b) * (exp_sum - S * N) / 2.0 * dt
    return np.float32(event_intensity - 1.0 * non_event)


def kernel(**inputs) -> np.ndarray:
    from concourse.bass_utils import run_bass_kernel_spmd

    nc = _build()
    in_maps, (t0, tn, E) = _marshal(inputs)
    res = run_bass_kernel_spmd(nc, in_maps, core_ids=list(range(NCORES)))
    beta = float(np.asarray(inputs["beta"]).reshape(-1)[0])
    out = _combine([r["out_p"] for r in res.results], beta, t0, tn, E)
    return np.asarray(out, dtype=np.float32)
